# revision 61
# baseline (speedup 1.0000x reference)
"""Trainium2 Bass kernel for sparse-attention AttnBlock.

The sparse gather-attention is rewritten as dense attention against a
host-built multiplicity matrix MT[j,q] = #{valid slots of query q hitting
key j}; softmax slots with equal indices merge into integer weights, so
W = MT * exp(K^T Q) reproduces the gather/mask/softmax path exactly with
pure matmuls (no on-device gather).

The device program is organized around the Activation engine's exp
throughput, the hard floor for this algorithm (16.8M exps/core at
1 elem/lane/cycle ~= 109us); measured ~116us/core:

  * one global 128-unit software pipeline: 4 per-head passes x 32
    key-chunks over all 1024 queries; two rotating [128,1024] score PSUM
    tiles; one wide exp and one plain [128,1024] mask-multiply (DVE 2x
    bf16) per unit; the AV matmuls trail two units so the PE never waits
    on the exp->multiply chain.
  * scores in bf16 (beats fp8 DoubleRow: DR's interleaved 256-col
    LDWEIGHTS with FWL disabled costs more than its halved matmul
    cycles at ~1k moving columns per stationary).  K stacks each
    head-pair vertically into one [128,128] stationary (head 2p on rows
    0-63, 2p+1 on 64-127) so LDWEIGHTS runs as a 128-row Fast Weight
    Load; the matching Q rows of the other head are zeroed, adding
    exactly nothing to the contraction.  HW matmul cost is
    ldweights(weight cols/1.2, /2.4 under FWL) + out-cols compute — the
    cost model's "TODO: model LD_WEIGHTS" hides this; measure on HW.
  * GroupNorm is FOLDED INTO THE WEIGHTS: h = s*x + b per channel, so
    q/k/v projections run on x with runtime-scaled weights ws = W*s and
    biases W@b + b_proj (tiny matmuls); the v-side bias routes through
    the softmax (sum w*(v+bv) = AV0 + bv*Z => at = AV0/Z + bv_tot) into
    the output bias via the host-shipped A = wo_p@wv block.  This kills
    h/hq SBUF tiles, 4 big DVE normalize ops, and 64 per-chunk v-bias
    matmuls; v-projection emits all 4 heads in one [128,256] PSUM tile.
  * PSUM budget exactly 8 banks: sc0/sc1 (2+2) + po [65,1024] (2) + 2
    proj/GN scratch; po is single-buffered per pass, its division
    deferred into the next pass's lead-in (units 2..7, after the
    trailing AVs flush).
  * k/v/q projections beyond the first chunks are emitted INTO the
    pipeline (deadline-ordered deferred list); out-projections need all
    four heads' divisions and run in the tail.
  * all 8 mt tiles ([128,4,1024] bf16 per 4-chunk block, 64KB) stay
    resident — the bf16 K-stack layout freed the partition space, and
    dropping the per-pass re-fetch removes 12MB/rep of DMA traffic.
  * GroupNorm stats: one-pass bn_stats on stride-2 subsampled x
    (estimator noise ~0.8% of sigma); rstd via exp(-0.5*ln(var+eps)) so
    ln/exp share one ACT table set (no swaps).
  * x/xq in bf16; residual quantization ~3e-3 of the output scale vs
    the 2e-2 gate.

Sharding: 8 cores = batch (2) x query-quarter (4); outputs concatenate.
Compiled executors are cached in _CACHE as jitted shard_map callables —
rebuilding/recompiling per call (the old run_bass_kernel_spmd path) costs
~1.5-30s per invocation and was what the original 98ms "HW exec time"
actually measured.
"""

import numpy as np
import ml_dtypes
from contextlib import ExitStack

import jax
from jax.sharding import Mesh, NamedSharding, PartitionSpec
from jax.experimental.shard_map import shard_map

import concourse.bass as bass
import concourse.bacc as bacc
import concourse.mybir as mybir
import concourse.tile as tile
from concourse.bass2jax import (
    _bass_exec_p, install_neuronx_cc_hook, partition_id_tensor)

B, C, HI, WI = 2, 256, 64, 64
NQ = HI * WI
HEADS, D = 4, 64
GROUPS, EPS = 32, 1e-6
NCORES = 8
QS = NQ // (NCORES // B)
NJC = NQ // 128
QH = 512
CPG = C // GROUPS
GPC = 128 // CPG

f32, bf16 = mybir.dt.float32, mybir.dt.bfloat16
f8 = mybir.dt.float8e4
DR = mybir.MatmulPerfMode.DoubleRow
FT = mybir.ActivationFunctionType
OP = mybir.AluOpType

_CACHE = {}
CF_COLS = 10 + GPC


def _split_dma_waits(nc):
    f = nc.m.functions[0]
    for bb in f.blocks:
        i = 0
        insts = bb.instructions
        while i < len(insts):
            ins = insts[i]
            si = ins.sync_info
            if (str(ins.opcode) in ("DMACopy", "DMATranspose")
                    and si is not None and len(si.on_wait) > 1):
                nop = mybir.InstNoOp(
                    name=nc.get_next_instruction_name(), ins=[], outs=[])
                nop.engine = ins.engine
                nop.sync_info = mybir.SyncInfo(
                    on_wait=list(si.on_wait)[:-1], on_update=[])
                si.on_wait = [si.on_wait[-1]]
                nc.register_instruction(nop)
                insts.insert(i, nop)
                i += 1
            i += 1


def _force_act_set(nc):
    """All ACT functions used here (Ln, Exp) live in one table set
    (natural_log_exp_and_others); force every load to it and drop the
    duplicates so the GN rstd doesn't thrash tables against the exps."""
    from concourse.hw_specs import get_activation_tables
    names = list(get_activation_tables(nc.m.arch).keys())
    want = names.index("natural_log_exp_and_others")
    for bb in nc.m.functions[0].blocks:
        first = True
        keep = []
        for ins in bb.instructions:
            if isinstance(ins, mybir.InstLoadActFuncSet):
                if not first:
                    # preserve any sync the dropped load carried
                    si = ins.sync_info
                    if si is not None and (si.on_wait or si.on_update):
                        nop = mybir.InstNoOp(
                            name=nc.get_next_instruction_name(), ins=[],
                            outs=[])
                        nop.engine = ins.engine
                        nop.sync_info = si
                        nc.register_instruction(nop)
                        keep.append(nop)
                    continue
                ins.act_func_set_id = want
                first = False
            keep.append(ins)
        bb.instructions[:] = keep


def _dedup_ldweights(nc):
    """Drop an Ldweights whose access pattern is identical to the previous
    one on the PE queue with only Matmults in between — the PE array still
    holds those weights (QK/AV emit per-unit matmul pairs sharing one
    stationary operand).  Sync carried by the dropped load moves to a NoOp."""
    def ap_key(ins):
        a = ins.ins[0]
        return (str(a.ap), a.offset, str(a.dtype),
                getattr(a, "memref", None) and str(a.memref))

    for bb in nc.m.functions[0].blocks:
        last_ld = None
        keep = []
        for ins in bb.instructions:
            op = str(ins.opcode)
            if str(getattr(ins, "engine", "")) != "EngineType.PE":
                keep.append(ins)
                continue
            if op == "Ldweights":
                k = ap_key(ins)
                if last_ld == k:
                    si = ins.sync_info
                    if si is not None and (si.on_wait or si.on_update):
                        nop = mybir.InstNoOp(
                            name=nc.get_next_instruction_name(), ins=[],
                            outs=[])
                        nop.engine = ins.engine
                        nop.sync_info = si
                        nc.register_instruction(nop)
                        keep.append(nop)
                    continue
                last_ld = k
            elif op != "Matmult":
                last_ld = None
            keep.append(ins)
        bb.instructions[:] = keep


def _build_program(stage="full", reps=1):
    nc = bacc.Bacc(None, target_bir_lowering=False, debug=False)
    _declare_and_emit(nc, stage, reps)
    nc.compile()
    _force_act_set(nc)
    _split_dma_waits(nc)
    return nc


def _declare_and_emit(nc, stage, reps=1):

    x_d = nc.dram_tensor("x", [2, 128, NQ], bf16, kind="ExternalInput")
    xq_d = nc.dram_tensor("xq", [2, 128, QS], bf16, kind="ExternalInput")
    mt_d = nc.dram_tensor("mt", [NQ, QS], bf16, kind="ExternalInput")
    wall_d = nc.dram_tensor("wall", [2, 128, 5 * C], bf16, kind="ExternalInput")
    cf_d = nc.dram_tensor("cf", [128, CF_COLS], f32, kind="ExternalInput")
    gselT_d = nc.dram_tensor("gselT", [GPC, 128], f32, kind="ExternalInput")
    out_d = nc.dram_tensor("out", [2, 128, QS], f32, kind="ExternalOutput")

    with tile.TileContext(nc) as tc, ExitStack() as ctx:
        const = ctx.enter_context(tc.tile_pool(name="const", bufs=1))
        big = ctx.enter_context(tc.tile_pool(name="big", bufs=1))
        sm = ctx.enter_context(tc.tile_pool(name="sm", bufs=2))
        mtp = ctx.enter_context(tc.tile_pool(name="mtp", bufs=1))
        pwp = ctx.enter_context(tc.tile_pool(name="pwp", bufs=2))
        psum = ctx.enter_context(
            tc.tile_pool(name="psum", bufs=1, space=bass.MemorySpace.PSUM))

        for _rep in range(reps):
            def big2(shape, dtype, tag):
                return [big.tile(shape, dtype, tag=f"{tag}{ci}", name=f"{tag}{ci}")
                        for ci in range(2)]

            x_sb = big2([128, NQ], bf16, "x")
            xq_sb = big2([128, QS], bf16, "xq")
            for ci in (1, 0):
                for hf in range(2):
                    nc.sync.dma_start(
                        out=x_sb[ci][:, hf * 2048:(hf + 1) * 2048],
                        in_=x_d[ci][:, hf * 2048:(hf + 1) * 2048])
                nc.sync.dma_start(out=xq_sb[ci][:], in_=xq_d[ci])

            wall = []
            for ci in range(2):
                t = const.tile([128, 5 * C], bf16, tag=f"wall{ci}", name=f"wall{ci}")
                nc.sync.dma_start(out=t[:], in_=wall_d[ci])
                wall.append(t)
            cf = const.tile([128, CF_COLS], f32, tag="cf", name="cf")
            nc.sync.dma_start(out=cf[:], in_=cf_d[:])
            gselT = const.tile([GPC, 128], f32, tag="gselT", name="gselT")
            nc.sync.dma_start(out=gselT[:], in_=gselT_d[:])
            wqT = [wall[ci][:, 0 * C:1 * C] for ci in range(2)]
            wkT = [wall[ci][:, 1 * C:2 * C] for ci in range(2)]
            wvT = [wall[ci][:, 2 * C:3 * C] for ci in range(2)]
            woT = [wall[ci][:, 3 * C:4 * C] for ci in range(2)]
            wovT = [wall[ci][:, 4 * C:5 * C] for ci in range(2)]
            bo = [cf[:, ci:ci + 1] for ci in range(2)]
            gam = [cf[:, 2 + ci:3 + ci] for ci in range(2)]
            bet = [cf[:, 4 + ci:5 + ci] for ci in range(2)]
            bq = [cf[:, 6 + ci:7 + ci] for ci in range(2)]
            bk = [cf[:, 8 + ci:9 + ci] for ci in range(2)]
            gsel = cf[:, 10:10 + GPC]

            # all 8 mt tiles stay resident (the bf16 K-stack layout freed
            # the partition space the old rotation scheme was buying)
            mtq_tiles = {}

            def emit_mtq(g):
                t = mtp.tile([128, 4, QS], bf16, tag=f"mtq{g}",
                             name=f"mtq{g}")
                src_ = mt_d[g * 512:(g + 1) * 512, :]
                nc.sync.dma_start(
                    out=t[:], in_=src_.rearrange("(c p) q -> p c q", c=4))
                mtq_tiles[g] = t

            for g in range(2):
                emit_mtq(g)

            epsb = const.tile([GPC, 1], f32, tag="epsb")
            nc.vector.memset(epsb[:], EPS)

            # GN affine is folded into the projection weights: h = s*x + b
            # per channel, so W@h = (W*s)@x + W@b.  ws holds the runtime
            # scaled q/k/v weights; the W@b terms fold into the biases.
            ws = [big.tile([128, 3 * C], bf16, tag=f"ws{ci}", name=f"ws{ci}")
                  for ci in range(2)]
            # bf16 K/Q — bf16 beats fp8 DoubleRow here: DR's interleaved
            # 256-col LDWEIGHTS (no FWL) costs more than its halved matmul
            # cycles when each stationary serves only ~1k moving columns.
            # K stacks each head-PAIR vertically into one [128,128]
            # stationary (head 2p on rows 0-63, head 2p+1 on 64-127) so the
            # 128-row weight triggers Fast Weight Load; the matching Q rows
            # of the other head are zero, so the extra contraction rows
            # contribute exactly nothing.
            k8 = big.tile([128, 2, NQ], bf16, tag="k8")
            q8 = big.tile([128, HEADS, QS], bf16, tag="q8")
            for i in range(2):
                nc.vector.memset(
                    q8[i * 64:(i + 1) * 64, (1 - i):(1 - i) + 3:2, :], 0.0)
            k8s = big2([128, NQ], bf16, "k8s")
            q8s = big2([128, QS], bf16, "q8s")
            vt_sb = big.tile([128, NJC * 260], bf16, tag="vt")
            at_sb = big2([128, QS], bf16, "at")
            out_sb = big2([128, QS], f32, "outs")

            base_stage, _, probes = stage.partition("-")
            PROBE = set(probes.split("+")) if probes else set()
            LVL = {"s0": 0, "s1": 1, "s2": 2, "s3": 3, "full": 6}[base_stage]

            if LVL == 0:
                for co in range(2):
                    nc.vector.tensor_copy(out_sb[co][:], xq_sb[co][:])
                for co in range(2):
                    nc.sync.dma_start(out=out_d[co], in_=out_sb[co][:])
                return

            # ================= GroupNorm =================
            scl_of, bias_of = {}, {}
            for ci in (1, 0):
                estat = sm.tile([128, 2], f32, tag="estat")
                bnb = sm.tile([128, 8 * 6], f32, tag="bnb")
                for c8 in range(8):
                    # stride-2 subsample: 256 of 512 per chunk — the
                    # estimator noise (~0.8% of sigma) is far inside the
                    # accuracy budget and halves the stats cost
                    xs = x_sb[ci][:, c8 * 512:(c8 + 1) * 512]
                    xs = xs.rearrange("p (a b) -> p b a", b=2)[:, 0, :]
                    nc.vector.bn_stats(bnb[:, c8 * 6:(c8 + 1) * 6], xs)
                mv = sm.tile([128, 2], f32, tag="mv")
                nc.vector.bn_aggr(mv[:], bnb[:])
                nc.vector.tensor_copy(estat[:, 0:1], mv[:, 0:1])
                nc.vector.scalar_tensor_tensor(
                    estat[:, 1:2], mv[:, 0:1], mv[:, 0:1], mv[:, 1:2],
                    op0=OP.mult, op1=OP.add)
                # gsel carries the 1/CPG scale (host-side), so gstat is the
                # per-group (mean, E[x^2]) directly
                gstat = psum.tile([GPC, 2], f32, tag="f0", name="gstat")
                nc.tensor.matmul(gstat[:], gsel, estat[:], start=True, stop=True)
                gm = sm.tile([GPC, 2], f32, tag="gm")
                nc.vector.tensor_copy(gm[:], gstat[:])
                var = sm.tile([GPC, 2], f32, tag="var")
                nc.vector.scalar_tensor_tensor(
                    var[:, 1:2], gm[:, 0:1], gm[:, 0:1], gm[:, 1:2],
                    op0=OP.mult, op1=OP.subtract)  # mean^2 - E[x^2] = -var
                # rstd = exp(-0.5*ln(var+eps)) — ln and exp share one ACT
                # table set, so no table swap against the attention exps.
                # ln input is -(-var) via scale=-1.
                lnv = sm.tile([GPC, 2], f32, tag="lnv")
                nc.scalar.activation(lnv[:, 0:1], var[:, 1:2], FT.Ln,
                                     bias=epsb[:], scale=-1.0)
                rs2 = sm.tile([GPC, 2], f32, tag="rs2")
                nc.scalar.activation(rs2[:, 1:2], lnv[:, 0:1], FT.Exp,
                                     scale=-0.5)
                nc.vector.tensor_copy(rs2[:, 0:1], gm[:, 0:1])
                chst = psum.tile([128, 2], f32, tag="f1", name="chst")
                nc.tensor.matmul(chst[:], gselT[:], rs2[:], start=True,
                                 stop=True)
                scl = sm.tile([128, 1], f32, tag="scl")
                nc.vector.tensor_tensor(scl[:], chst[:, 1:2], gam[ci], OP.mult)
                mscl = sm.tile([128, 1], f32, tag="mscl")
                nc.vector.tensor_tensor(mscl[:], chst[:, 0:1], scl[:], OP.mult)
                bias_c = sm.tile([128, 1], f32, tag="biasc")
                nc.vector.tensor_tensor(bias_c[:], bet[ci], mscl[:], OP.subtract)
                bias_b = sm.tile([128, 1], bf16, tag="biasb")
                nc.vector.tensor_copy(bias_b[:], bias_c[:])
                scl_of[ci], bias_of[ci] = scl, bias_b
                # scaled q/k/v weights for this input-channel chunk
                nc.vector.tensor_scalar(
                    ws[ci][:, 0:3 * C], wall[ci][:, 0:3 * C],
                    scl[:], None, op0=OP.mult)

            # fold W@bias_c into the projection biases: per-co [128,1]
            # columns for q/k (added post-PSUM); the v-side bias routes
            # through the softmax into the output bias via A = wo_p@wv
            # (wall's 5th block), since at = AV0/Z + (wv@bias_c + bv).
            pbqk = psum.tile([128, 6], f32, tag="f0", name="pbqk")
            for j, pj in enumerate((0, 1, 4)):
                for co in range(2):
                    for ci in range(2):
                        nc.tensor.matmul(
                            pbqk[:, 2 * j + co:2 * j + co + 1],
                            wall[ci][:, pj * C + co * 128:
                                     pj * C + (co + 1) * 128],
                            bias_of[ci][:], start=(ci == 0), stop=(ci == 1))
            bqp, bkp, bo2 = [], [], []
            for co in range(2):
                t = sm.tile([128, 1], f32, tag=f"bqp{co}")
                nc.vector.tensor_tensor(t[:], pbqk[:, co:co + 1], bq[co],
                                        OP.add)
                bqp.append(t)
                t = sm.tile([128, 1], f32, tag=f"bkp{co}")
                nc.vector.tensor_tensor(t[:], pbqk[:, 2 + co:3 + co], bk[co],
                                        OP.add)
                bkp.append(t)
                t = sm.tile([128, 1], f32, tag=f"bo2{co}")
                nc.vector.tensor_tensor(t[:], pbqk[:, 4 + co:5 + co], bo[co],
                                        OP.add)
                bo2.append(t)

            if LVL == 1:
                for co in range(2):
                    nc.vector.tensor_copy(out_sb[co][:], xq_sb[co][:])
                for co in range(2):
                    nc.sync.dma_start(out=out_d[co], in_=out_sb[co][:])
                return

            # ================= Projection emitters =================
            _fidx = [0]

            def ftag():
                _fidx[0] ^= 1
                return f"f{_fidx[0]}"

            def emit_qproj(co, s):
                pq = psum.tile([128, 512], f32, tag=ftag(), name="pq")
                for ci in range(2):
                    nc.tensor.matmul(
                        pq[:], ws[ci][:, co * 128:(co + 1) * 128],
                        xq_sb[ci][:, s * 512:(s + 1) * 512],
                        start=(ci == 0), stop=(ci == 1))
                cols = slice(s * 512, (s + 1) * 512)
                nc.vector.tensor_scalar(
                    q8s[co][:, cols], pq[:], bqp[co][:], None, op0=OP.add)
                for i in range(2):
                    nc.sync.dma_start(
                        out=q8[i * 64:(i + 1) * 64, 2 * co + i, cols],
                        in_=q8s[co][i * 64:(i + 1) * 64, cols])

            def emit_kproj(co, s):
                pk = psum.tile([128, 512], f32, tag=ftag(), name="pk")
                for ci in range(2):
                    nc.tensor.matmul(
                        pk[:], ws[ci][:, C + co * 128:C + (co + 1) * 128],
                        x_sb[ci][:, s * 512:(s + 1) * 512],
                        start=(ci == 0), stop=(ci == 1))
                cols = slice(s * 512, (s + 1) * 512)
                nc.vector.tensor_scalar(
                    k8s[co][:, cols], pk[:], bkp[co][:], None, op0=OP.add)
                if s % 2 == 1:
                    qcols = slice((s - 1) * 512, (s + 1) * 512)
                    for i in range(2):
                        nc.sync.dma_start(
                            out=k8[i * 64:(i + 1) * 64, co, qcols],
                            in_=k8s[co][i * 64:(i + 1) * 64, qcols])

            def emit_vproj(jc):
                # all 4 heads at once; v-bias is folded into the output
                # bias (bo2), so only the two x-chunk matmuls remain
                pv = psum.tile([128, 256], f32, tag=ftag(), name="pv")
                for ci in range(2):
                    nc.tensor.matmul(
                        pv[:], x_sb[ci][:, jc * 128:(jc + 1) * 128],
                        ws[ci][:, 2 * C:3 * C],
                        start=(ci == 0), stop=(ci == 1))
                base = jc * 260
                vt_view = vt_sb[:, base:base + 260].rearrange(
                    "p (h c) -> p h c", h=4)[:, :, 0:64]
                pv_view = pv[:].rearrange("p (h c) -> p h c", h=4)
                nc.vector.tensor_copy(vt_view, pv_view)

            vt_ones = vt_sb.rearrange(
                "p (j h c) -> p j h c", j=NJC, h=HEADS)[:, :, :, 64:65]
            nc.vector.memset(vt_ones, 1.0)

            # serial front: only what the first scores/AV need
            emit_qproj(0, 0)
            emit_qproj(0, 1)
            emit_kproj(0, 0)
            emit_kproj(0, 1)
            emit_vproj(0)
            emit_vproj(1)

            # deferred projection work, deadline-ordered in GLOBAL unit index
            # (passes are per-head, 32 units).  k(co,s) is read by scores at
            # unit 4s of pass 2co; q(co,*) at pass 2co start; vt(jc,pr) by AV
            # at unit jc+1 of pass 2pr.
            def make_deferred():
                work = []
                for s in range(2, 8):
                    work.append((max(0, 4 * s - 6), "k", (0, s)))
                for s in range(8):
                    work.append((33 + 3 * s, "k", (1, s)))
                work.append((56, "q", (1, 0)))
                work.append((58, "q", (1, 1)))
                for jc in range(2, NJC):
                    work.append((jc - 1, "v", (jc,)))
                work.sort(key=lambda w: w[0])
                return work

            if LVL in (2, 3):
                for _d, kind, a in make_deferred():
                    (emit_vproj if kind == "v" else
                     emit_kproj if kind == "k" else emit_qproj)(*a)
                src = xq_sb if LVL == 2 else [
                    vt_sb[:, 0:1024], vt_sb[:, 1024:2048]]
                for co in range(2):
                    nc.vector.tensor_copy(out_sb[co][:], src[co][:])
                for co in range(2):
                    nc.sync.dma_start(out=out_d[co], in_=out_sb[co][:])
                return

            # ============ Attention: 4 per-head passes over all queries ======
            deferred = make_deferred()
            pending = []  # division/out-proj closures, run in next pass's lead

            def make_div(h, po_t):
                co, i = h // 2, h % 2

                def emit():
                    rz = sm.tile([1, QS], f32, tag="rz")
                    nc.vector.reciprocal(rz[:], po_t[64:65, :])
                    # broadcast 1/Z across the 64 head dims on the idle Pool
                    # engine (keeps the PE, which drops to low pstate in the
                    # tail, out of the division chain entirely)
                    bc = sm.tile([64, QS], f32, tag="bc")
                    nc.gpsimd.partition_broadcast(bc[:], rz[:])
                    nc.vector.tensor_tensor(
                        at_sb[co][i * 64:i * 64 + 64, :],
                        po_t[0:64, :], bc[:], OP.mult)
                return emit

            def make_outproj(co, qlo_):
                def emit():
                    pout = psum.tile([128, 512], f32, tag=ftag(), name="pout")
                    for ci in range(2):
                        nc.tensor.matmul(
                            pout[:], woT[ci][:, co * 128:(co + 1) * 128],
                            at_sb[ci][:, qlo_:qlo_ + QH],
                            start=(ci == 0), stop=(ci == 1))
                    nc.vector.scalar_tensor_tensor(
                        out_sb[co][:, qlo_:qlo_ + QH], pout[:], bo2[co][:],
                        xq_sb[co][:, qlo_:qlo_ + QH],
                        op0=OP.add, op1=OP.add)
                    nc.sync.dma_start(
                        out=out_d[co][:, qlo_:qlo_ + QH],
                        in_=out_sb[co][:, qlo_:qlo_ + QH])
                return emit

            # single global pipeline over 128 units (4 head-passes x 32)
            av_pending = []   # (due_gu, pass_idx, jc, wt_tile)
            po_of = {}        # pass_idx -> po tile

            avw = 8 if "tinyav" in PROBE else QS

            def flush_av(gu, force=False):
                while av_pending and (force or av_pending[0][0] <= gu):
                    _due, pi, pjc, pwt = av_pending.pop(0)
                    base = pjc * 260
                    for qh in range(2):
                        w = min(avw, QH)
                        nc.tensor.matmul(
                            po_of[pi][:, qh * QH:qh * QH + w],
                            vt_sb[:, base + pi * 65: base + pi * 65 + 65],
                            pwt[:, qh * QH:qh * QH + w],
                            start=(pjc == 0), stop=(pjc == NJC - 1))

            for gu in range(4 * NJC):
                p, u = gu // NJC, gu % NJC
                h = p
                jc = u
                if u == 0:
                    po_of[p] = psum.tile([65, QS], f32, tag="po", name=f"po{p}")
                ps = psum.tile([128, QS], f32, tag=f"sc{u % 2}", name="ps")
                qkw = 16 if "tinyqk" in PROBE else QH
                for qh in range(2):
                    nc.tensor.matmul(
                        ps[:, qh * QH:qh * QH + qkw],
                        k8[:, h // 2, jc * 128:(jc + 1) * 128],
                        q8[:, h, qh * QH:qh * QH + qkw],
                        start=True, stop=True)
                pt = pwp.tile([128, QS], bf16, tag="p", bufs=3)
                expw = 8 if "tinyexp" in PROBE else QS
                nc.scalar.activation(pt[:, 0:expw], ps[:, 0:expw], FT.Exp)
                flush_av(gu)
                if 2 <= u < 8 and pending:
                    pending.pop(0)()
                if p == 0 and u % 4 == 2:
                    g = u // 4 + 2
                    if g < 8:
                        emit_mtq(g)
                if deferred:
                    popped = 0
                    while deferred and (popped == 0
                                        or deferred[0][0] <= gu + 2):
                        if popped >= 2:
                            break
                        _d, kind, a = deferred.pop(0)
                        if kind == "v":
                            emit_vproj(*a)
                        elif kind == "k":
                            emit_kproj(*a)
                        else:
                            emit_qproj(*a)
                        popped += 1
                wt_t = pwp.tile([128, QS], bf16, tag="w", bufs=3)
                mt_v = mtq_tiles[jc // 4][:, jc % 4, :]
                mulw = 8 if "tinymul" in PROBE else QS
                nc.vector.tensor_tensor(
                    wt_t[:, 0:mulw], pt[:, 0:mulw], mt_v[0:128, 0:mulw],
                    OP.mult)
                av_pending.append((gu + 2, p, jc, wt_t))
                if u == NJC - 1:
                    # queue this pass's division for the next pass's lead-in
                    # units; out-projections need all four heads' divisions,
                    # so they run in the tail
                    pending.append(make_div(h, po_of[p]))
                    if h == 3:
                        for co in range(2):
                            for qh in range(2):
                                pending.append(make_outproj(co, qh * QH))

            flush_av(4 * NJC + 2, force=True)
            while pending:
                pending.pop(0)()


def _prep_in_maps(x, valid_indices_mask, attendable_indices, gn_scale, gn_bias,
                  wq, bq, wk, bk, wv, bv, wo, bo):
    x = np.ascontiguousarray(
        np.asarray(x, np.float32).reshape(B, C, NQ)).astype(ml_dtypes.bfloat16)
    idx = np.asarray(attendable_indices, np.int64)
    msk = np.asarray(valid_indices_mask, np.int64)

    qcol = np.arange(NQ, dtype=np.int64)[:, None]
    flat = ((idx * NQ + qcol).ravel())[msk.ravel() != 0]
    cnt = np.bincount(flat, minlength=NQ * NQ)
    MT = cnt.reshape(NQ, NQ).astype(ml_dtypes.bfloat16)

    def chunk_w(w):
        return np.ascontiguousarray(
            np.asarray(w, np.float32).T.reshape(2, 128, C)
        ).astype(ml_dtypes.bfloat16)

    def col_b(b):
        return np.asarray(b, np.float32).reshape(2, 128)

    gsel = np.zeros((128, GPC), np.float32)
    gsel[np.arange(128), np.arange(128) // CPG] = 1.0

    r = np.arange(C)
    perm = (r % D) * HEADS + (r // D)
    wo_p = np.asarray(wo, np.float32)[:, perm]

    # v-bias folds through the softmax (sum w*(v+bv) = AV0 + bv*Z, so
    # at = AV0/Z + bv_total) into the output projection:
    # out += wo_p @ (wv @ bias_c) + wo_p @ bv, with bias_c the on-device GN
    # shift.  A = wo_p @ wv ships with the weights; wo_p @ bv + bo becomes
    # the effective output bias.
    A = wo_p @ np.asarray(wv, np.float32)
    bo_eff = np.asarray(bo, np.float32) + wo_p @ np.asarray(bv, np.float32)

    wall = np.concatenate(
        [chunk_w(wq), chunk_w(wk), chunk_w(wv), chunk_w(wo_p), chunk_w(A)],
        axis=2)
    cf = np.zeros((128, CF_COLS), np.float32)
    for i, b in enumerate([bo_eff, gn_scale, gn_bias, bq, bk]):
        cb = col_b(b)
        cf[:, 2 * i] = cb[0]
        cf[:, 2 * i + 1] = cb[1]
    cf[:, 10:10 + GPC] = gsel / CPG

    shared = {
        "wall": np.ascontiguousarray(wall),
        "cf": cf,
        "gselT": np.ascontiguousarray(gsel.T),
    }
    in_maps = []
    for core in range(NCORES):
        b = core // (NCORES // B)
        qoff = (core % (NCORES // B)) * QS
        xb = x[b].reshape(2, 128, NQ)
        m = dict(shared)
        m["x"] = np.ascontiguousarray(xb)
        m["xq"] = np.ascontiguousarray(xb[:, :, qoff:qoff + QS])
        m["mt"] = np.ascontiguousarray(MT[:, qoff:qoff + QS])
        in_maps.append(m)
    return in_maps


class _Runner:
    """Holds the jitted 8-core executor for one compiled program so repeated
    calls skip XLA/neuronx recompilation (the old run_bass_kernel_spmd path
    rebuilt + recompiled the pjit callable on every invocation)."""

    def __init__(self, nc):
        install_neuronx_cc_hook()
        pname = nc.partition_id_tensor.name if nc.partition_id_tensor else None
        in_names, out_names, out_avals, zero_outs = [], [], [], []
        for alloc in nc.m.functions[0].allocations:
            if not isinstance(alloc, mybir.MemoryLocationSet):
                continue
            name = alloc.memorylocations[0].name
            if alloc.kind == "ExternalInput":
                if name != pname:
                    in_names.append(name)
            elif alloc.kind == "ExternalOutput":
                shape = tuple(alloc.tensor_shape)
                dtype = mybir.dt.np(alloc.dtype)
                out_names.append(name)
                out_avals.append(jax.core.ShapedArray(shape, dtype))
                zero_outs.append(np.zeros(shape, dtype))
        self.in_names, self.out_names = in_names, out_names
        self.out_avals = out_avals
        n_params, n_outs = len(in_names), len(out_avals)
        all_in = list(in_names) + list(out_names)
        if pname is not None:
            all_in.append(pname)

        def _body(*args):
            operands = list(args)
            if pname is not None:
                operands.append(partition_id_tensor())
            return tuple(_bass_exec_p.bind(
                *operands,
                out_avals=tuple(out_avals),
                in_names=tuple(all_in),
                out_names=tuple(out_names),
                lowering_input_output_aliases=(),
                sim_require_finite=True,
                sim_require_nnan=True,
                nc=nc,
            ))

        devices = jax.devices()[:NCORES]
        self.mesh = Mesh(np.asarray(devices), ("core",))
        in_specs = (PartitionSpec("core"),) * (n_params + n_outs)
        out_specs = (PartitionSpec("core"),) * n_outs
        self.fn = jax.jit(
            shard_map(_body, mesh=self.mesh, in_specs=in_specs,
                      out_specs=out_specs, check_rep=False),
            keep_unused=True)
        self.sharding = NamedSharding(self.mesh, PartitionSpec("core"))
        self.zero_outs = [
            jax.device_put(
                np.zeros((NCORES * z.shape[0], *z.shape[1:]), z.dtype),
                self.sharding)
            for z in zero_outs]

    def prep(self, in_maps):
        concat = [
            np.concatenate([np.asarray(in_maps[c][nm])
                            for c in range(NCORES)], axis=0)
            for nm in self.in_names]
        return [jax.device_put(a, self.sharding) for a in concat]

    def run(self, dev_in):
        outs = self.fn(*dev_in, *self.zero_outs)
        jax.block_until_ready(outs)
        return outs

    def results(self, outs):
        res = []
        for c in range(NCORES):
            m = {}
            for i, nm in enumerate(self.out_names):
                a = np.asarray(outs[i])
                m[nm] = a.reshape(NCORES, *self.out_avals[i].shape)[c]
            res.append(m)
        return res


class _Res:
    def __init__(self, results):
        self.results = results


def _get_runner(stage="full", reps=1):
    key = f"run_{stage}_{reps}"
    if key not in _CACHE:
        _CACHE[key] = _Runner(_build_program(stage, reps))
    return _CACHE[key]


def _execute(in_maps, trace=False, stage="full", reps=1):
    r = _get_runner(stage, reps)
    outs = r.run(r.prep(in_maps))
    return _Res(r.results(outs))


def _assemble(results):
    out = np.zeros((B, C, NQ), np.float32)
    for core in range(NCORES):
        b = core // (NCORES // B)
        qoff = (core % (NCORES // B)) * QS
        o = results[core]["out"]
        out[b, :, qoff:qoff + QS] = o.reshape(C, QS)
    return out.reshape(B, C, HI, WI)


def kernel(**inputs):
    in_maps = _prep_in_maps(**inputs)
    res = _execute(in_maps, trace=False)
    return _assemble(res.results)

